# revision 12
# baseline (speedup 1.0000x reference)
"""CMXBlock (dense transformer block) Trainium2 Bass kernel.

Sharding: data-parallel over batch B=8 across the 8 NeuronCores — one image
per core, all weights replicated, no collectives.

Per-core computation (C=256 channels on partitions, HW=1024 positions free):
  x1 <- x1 + proj(softmax((q_w@bn1(x1))^T (k_w@bn1(x2)) * temp) @ (v_w@bn1(x2))^T)
  x1 <- x1 + fc2(gelu(dwconv3x3(fc1(bn2(x1)))))

Implementation notes:
 - BatchNorms are folded into the 1x1-conv weights host-side (bias applied
   per-partition at PSUM eviction).  temp is folded into q_w rows.
 - Matmuls run in float32r (full PE rate, ~1.8e-4 rel err) except the
   attention AV, depthwise conv and fc2 which run in bf16 (their moving
   operands are produced by ACT/DVE which cannot emit float32r).
 - Softmax: scores are built transposed (S^T[m,n] per m-tile), exp on ACT
   (no max subtraction — scores are bounded, fp32/bf16 range is ample), the
   denominator comes from an extra all-ones column in the AV stationary
   operand, and the 1/Z column scale is applied via a gpsimd
   partition-broadcast + one DVE multiply per channel half.
 - Depthwise 3x3 runs on the PE as 9 diagonal-matrix taps over an x-padded
   [32, 36] spatial layout so every shifted tap is a contiguous 1-D slice.
"""
import numpy as np

import concourse.bass as bass
import concourse.tile as tile
import concourse.mybir as mybir
from concourse import bacc
from concourse.bass_utils import run_bass_kernel_spmd

F32 = mybir.dt.float32
F32R = mybir.dt.float32r
BF16 = mybir.dt.bfloat16
AF = mybir.ActivationFunctionType
ALU = mybir.AluOpType

B, C, H, W = 8, 256, 32, 32
NH, DH = 8, 32          # heads, head dim
HW = H * W              # 1024 positions
HID = 4 * C             # 1024 mlp hidden channels
EPS = 1e-5
WP = W + 4              # x-padded row width (36, even)
PADF = H * WP           # padded flat spatial size (1152)
N_CORES = 8

_NC_CACHE = {}


def _tap_chunks(shift):
    """Bank-aligned (<=512) even-aligned chunks of a dw tap's dst range.

    float32r matmuls need even dst offsets and even moving counts; the
    elements dropped by the even-alignment are always x-pad columns (never
    read downstream), and the full-coverage center tap has written them.
    """
    lo, hi = max(0, -shift), min(PADF, PADF - shift)
    out = []
    for b0 in range(0, PADF, 512):
        a, b = max(lo, b0), min(hi, b0 + 512)
        a += a % 2
        n = (b - a) & ~1
        if n > 0:
            out.append((a, n))
    return out


def _build_body(nc, tc, io):
    x1d, x2d = io["x1"], io["x2"]
    outd = io["out"]

    import contextlib
    ctx = contextlib.ExitStack()
    with ctx:
        wpool = ctx.enter_context(tc.tile_pool(name="weights", bufs=1))
        # pB: attention results that survive until after proj
        pB = ctx.enter_context(tc.tile_pool(name="pB", bufs=1))

        # ---------- persistent SBUF tensors ----------
        x1 = wpool.tile([128, 2, HW], F32R, tag="x1")
        nc.sync.dma_start(x1[:], x1d[:])

        def wload(name, shape, dt):
            t = wpool.tile(shape, dt, tag=name)
            nc.sync.dma_start(t[:], io[name][:])
            return t

        qT = wload("qT", [128, 2, C], F32R)
        kT = wload("kT", [128, 2, C], F32R)
        vwT = wload("vwT", [128, 2, C], F32R)
        projT = wload("projT", [128, 2, C], F32R)
        fc1T = wload("fc1T", [128, 2, HID], F32R)
        fc2T = wload("fc2T", [128, 8, C], F32R)
        qb = wload("qb", [128, 2], F32)
        kb = wload("kb", [128, 2], F32)
        projb = wload("projb", [128, 2], F32)
        fc1b = wload("fc1b", [128, 8], F32)
        fc2b = wload("fc2b", [128, 2], F32)
        inv1 = wload("inv1", [128, 2], F32)
        beta1 = wload("beta1", [128, 2], F32)

        attnU = pB.tile([128, 2, HW], F32, tag="attnU")
        z8 = pB.tile([NH, HW], F32, tag="z8")

        with tc.tile_pool(name="pA", bufs=1) as pA:
            q_sb = pA.tile([128, 2, HW], F32R, tag="q")
            k_sb = pA.tile([128, 2, HW], F32R, tag="k")
            vt1 = pA.tile([128, 8, NH, DH + 1], F32R, tag="vt1")  # [p, mt, h, v|1]
            # the all-ones denominator column must come from a DVE op (the
            # only engine the verifier accepts as a float32r producer)
            onesf = pA.tile([128, 8 * NH], F32, tag="onesf")
            nc.gpsimd.memset(onesf[:], 1.0)
            nc.vector.tensor_copy(
                vt1[:, :, :, DH:DH + 1],
                onesf[:].rearrange("p (a b c) -> p a b c", a=8, b=NH))

            # ---------- phase 1: q, k projections; x2n; v^T ----------
            with tc.tile_pool(name="p1", bufs=1) as p1, \
                 tc.tile_pool(name="ps1", bufs=2, space="PSUM") as ps1:
                x2 = p1.tile([128, 2, HW], F32R, tag="x2")
                nc.sync.dma_start(x2[:], x2d[:])
                for (wT, bias, dst) in ((qT, qb, q_sb), (kT, kb, k_sb)):
                    rhs = x1 if dst is q_sb else x2
                    for mt in range(2):
                        for chk in range(2):
                            ps = ps1.tile([128, 512], F32, tag="mm")
                            for kt in range(2):
                                nc.tensor.matmul(
                                    ps[:], wT[:, kt, 128 * mt:128 * (mt + 1)],
                                    rhs[:, kt, 512 * chk:512 * (chk + 1)],
                                    start=(kt == 0), stop=(kt == 1))
                            nc.vector.tensor_scalar_add(
                                dst[:, mt, 512 * chk:512 * (chk + 1)], ps[:],
                                bias[:, mt:mt + 1])

                x2n = p1.tile([128, 2, HW], F32R, tag="x2n")
                for kt in range(2):
                    nc.vector.tensor_scalar(
                        x2n[:, kt, :], x2[:, kt, :],
                        inv1[:, kt:kt + 1], beta1[:, kt:kt + 1], ALU.mult, ALU.add)

                for mp in range(8):
                    ps = ps1.tile([128, 512], F32, tag="mm")
                    for kt in range(2):
                        nc.tensor.matmul(
                            ps[:, 0:C], x2n[:, kt, 128 * mp:128 * (mp + 1)],
                            vwT[:, kt, :], start=(kt == 0), stop=(kt == 1))
                    nc.vector.tensor_copy(
                        vt1[:, mp, :, 0:DH],
                        ps[:, 0:C].rearrange("p (h d) -> p h d", h=NH))

            # ---------- phase 2: attention per head ----------
            expS = pA.tile([128, 8, HW], F32R, tag="expS")
            with tc.tile_pool(name="ef", bufs=3) as epool, \
                 tc.tile_pool(name="uz", bufs=2) as uz_pool, \
                 tc.tile_pool(name="pss", bufs=2, space="PSUM") as pss, \
                 tc.tile_pool(name="psa", bufs=2, space="PSUM") as psa:
                for h in range(NH):
                    hb, half = 32 * (h % 4), h // 4
                    ps_av = psa.tile([DH + 1, HW], F32, tag="av")
                    pending = None  # mt awaiting AV
                    for mt in range(8):
                        ps_s = pss.tile([128, HW], F32, tag="s")
                        for chk in range(2):
                            nc.tensor.matmul(
                                ps_s[:, 512 * chk:512 * (chk + 1)],
                                k_sb[hb:hb + 32, half, 128 * mt:128 * (mt + 1)],
                                q_sb[hb:hb + 32, half, 512 * chk:512 * (chk + 1)],
                                start=True, stop=True, tile_position=(hb, 0))
                        ef = epool.tile([128, HW], F32, tag="ef")
                        nc.scalar.activation(ef[:], ps_s[:], AF.Exp)
                        nc.gpsimd.tensor_copy(expS[:, mt, :], ef[:])
                        if pending is not None:
                            pmt = pending
                            for chk in range(2):
                                nc.tensor.matmul(
                                    ps_av[:, 512 * chk:512 * (chk + 1)],
                                    vt1[:, pmt, h, :],
                                    expS[:, pmt, 512 * chk:512 * (chk + 1)],
                                    start=(pmt == 0), stop=False)
                        pending = mt
                    for chk in range(2):
                        nc.tensor.matmul(
                            ps_av[:, 512 * chk:512 * (chk + 1)],
                            vt1[:, 7, h, :],
                            expS[:, 7, 512 * chk:512 * (chk + 1)],
                            start=False, stop=True)
                    uz = uz_pool.tile([DH + 1, HW], F32, tag="uz")
                    nc.vector.tensor_copy(uz[:], ps_av[:])
                    nc.sync.dma_start(z8[h:h + 1, :], uz[DH:DH + 1, :])
                    nc.sync.dma_start(attnU[hb:hb + 32, half, :], uz[0:DH, :])

        # ---------- phase 3: normalize, proj, residual ----------
        r8 = pB.tile([NH, HW], F32, tag="r8")
        nc.vector.reciprocal(r8[:], z8[:])
        rbc = pB.tile([128, 2, HW], F32, tag="rbc")
        with tc.tile_pool(name="rtmp", bufs=2) as rpool:
            for h in range(NH):
                hb, half = 32 * (h % 4), h // 4
                rt = rpool.tile([1, HW], F32, tag="rt")
                nc.sync.dma_start(rt[:], r8[h:h + 1, :])
                # partition_broadcast always writes from partition 0, so
                # broadcast into a base-0 temp and DMA to the head's rows.
                rb = rpool.tile([32, HW], F32, tag="rb")
                nc.gpsimd.partition_broadcast(rb[:], rt[:])
                nc.sync.dma_start(rbc[hb:hb + 32, half, :], rb[:])
        attn_r = pB.tile([128, 2, HW], F32R, tag="attn_r")
        for half in range(2):
            nc.vector.tensor_mul(attn_r[:, half, :], attnU[:, half, :],
                                 rbc[:, half, :])

        with tc.tile_pool(name="pC", bufs=1) as pC, \
             tc.tile_pool(name="ps2", bufs=2, space="PSUM") as ps2, \
             tc.tile_pool(name="psd", bufs=2, space="PSUM") as psd:
            x1u = pC.tile([128, 2, HW], F32R, tag="x1u")
            h1 = pC.tile([128, 8, H, WP], F32R, tag="h1")
            # pad columns must come from a DVE op (f32r-producer whitelist)
            zpad = pC.tile([128, 8 * H * 2], F32, tag="zpad")
            nc.gpsimd.memset(zpad[:], 0.0)
            zsrc = zpad[:].rearrange("p (c a b) -> p c a b", c=8, a=H)
            nc.vector.tensor_copy(h1[:, :, :, 0:2], zsrc)
            nc.vector.tensor_copy(h1[:, :, :, WP - 2:WP], zsrc)
            hgr = pC.tile([128, 8, HW], F32R, tag="hgr")
            out_sb = pC.tile([128, 2, HW], F32, tag="out")

            # proj + residual1
            for mt in range(2):
                for chk in range(2):
                    ps = ps2.tile([128, 512], F32, tag="mm")
                    for kt in range(2):
                        nc.tensor.matmul(
                            ps[:], projT[:, kt, 128 * mt:128 * (mt + 1)],
                            attn_r[:, kt, 512 * chk:512 * (chk + 1)],
                            start=(kt == 0), stop=(kt == 1))
                    nc.vector.scalar_tensor_tensor(
                        x1u[:, mt, 512 * chk:512 * (chk + 1)], ps[:],
                        projb[:, mt:mt + 1],
                        x1[:, mt, 512 * chk:512 * (chk + 1)],
                        ALU.add, ALU.add)

            # ---------- phase 4: MLP ----------
            for mt in range(8):
                for chk in range(2):
                    ps = ps2.tile([128, 512], F32, tag="mm")
                    for kt in range(2):
                        nc.tensor.matmul(
                            ps[:], fc1T[:, kt, 128 * mt:128 * (mt + 1)],
                            x1u[:, kt, 512 * chk:512 * (chk + 1)],
                            start=(kt == 0), stop=(kt == 1))
                    nc.vector.tensor_scalar_add(
                        h1[:, mt, 16 * chk:16 * (chk + 1), 2:W + 2],
                        ps[:].rearrange("p (a b) -> p a b", a=16),
                        fc1b[:, mt:mt + 1])

            # depthwise 3x3 (9 diagonal taps, f32r, diag tiles streamed) + gelu
            h1f = h1[:].rearrange("p c a b -> p c (a b)")
            taps = [(dy, dx) for dy in (-1, 0, 1) for dx in (-1, 0, 1)]
            taps.remove((0, 0))
            taps = [(0, 0)] + taps      # center first: full coverage, start=True
            with tc.tile_pool(name="dwd", bufs=4) as dpool, \
                 tc.tile_pool(name="hgf", bufs=2) as gpool:
                for ct in range(8):
                    ps_dw = psd.tile([128, PADF], F32, tag="dw")
                    for ti, (dy, dx) in enumerate(taps):
                        dwt = dpool.tile([128, 128], F32R, tag="dwt")
                        nc.sync.dma_start(
                            dwt[:], io["dwd"][ct, 3 * (dy + 1) + (dx + 1)])
                        shift = dy * WP + dx
                        chunks = _tap_chunks(shift)
                        for ci, (c0, n) in enumerate(chunks):
                            nc.tensor.matmul(
                                ps_dw[:, c0:c0 + n],
                                dwt[:],
                                h1f[:, ct, c0 + shift:c0 + shift + n],
                                start=(ti == 0),
                                stop=(ti == len(taps) - 1 and ci == len(chunks) - 1))
                    hgf = gpool.tile([128, HW], F32, tag="hgf")
                    nc.scalar.activation(
                        hgf[:],
                        ps_dw[:].rearrange("p (a b) -> p a b", a=H)[:, :, 2:W + 2],
                        AF.Gelu)
                    nc.gpsimd.tensor_copy(hgr[:, ct, :], hgf[:])

            # fc2 + residual2
            for mt in range(2):
                for chk in range(2):
                    ps = ps2.tile([128, 512], F32, tag="mm")
                    for kt in range(8):
                        nc.tensor.matmul(
                            ps[:], fc2T[:, kt, 128 * mt:128 * (mt + 1)],
                            hgr[:, kt, 512 * chk:512 * (chk + 1)],
                            start=(kt == 0), stop=(kt == 7))
                    nc.vector.scalar_tensor_tensor(
                        out_sb[:, mt, 512 * chk:512 * (chk + 1)], ps[:],
                        fc2b[:, mt:mt + 1],
                        x1u[:, mt, 512 * chk:512 * (chk + 1)],
                        ALU.add, ALU.add)

            nc.sync.dma_start(outd[:], out_sb[:])


def _build_nc():
    if "nc" in _NC_CACHE:
        return _NC_CACHE["nc"]
    nc = bacc.Bacc(trn_type="TRN2", target_bir_lowering=False, debug=False)
    io = {}
    for name, shape, dt in [
        ("x1", [128, 2, HW], F32R), ("x2", [128, 2, HW], F32R),
        ("qT", [128, 2, C], F32R), ("kT", [128, 2, C], F32R),
        ("vwT", [128, 2, C], F32R), ("projT", [128, 2, C], F32R),
        ("fc1T", [128, 2, HID], F32R), ("fc2T", [128, 8, C], F32R),
        ("dwd", [8, 9, 128, 128], F32R),
        ("qb", [128, 2], F32), ("kb", [128, 2], F32), ("projb", [128, 2], F32),
        ("fc1b", [128, 8], F32), ("fc2b", [128, 2], F32),
        ("inv1", [128, 2], F32), ("beta1", [128, 2], F32),
    ]:
        io[name] = nc.dram_tensor(name, shape, dt, kind="ExternalInput").ap()
    io["out"] = nc.dram_tensor("out", [128, 2, HW], F32, kind="ExternalOutput").ap()

    with tile.TileContext(nc) as tc:
        _build_body(nc, tc, io)
    nc.compile()
    _NC_CACHE["nc"] = nc
    return nc


def _to_part_layout(a, ntiles):
    """[ntiles*128, F] -> [128, ntiles, F] with c = kt*128 + p."""
    return np.ascontiguousarray(
        a.reshape(ntiles, 128, -1).transpose(1, 0, 2))


def _bias_layout(b, ntiles):
    """[ntiles*128] -> [128, ntiles]."""
    return np.ascontiguousarray(b.reshape(ntiles, 128).T)


def _prepare_weights(bn1_g, bn1_b, bn1_m, bn1_v, q_w, k_w, v_w, temp, proj_w,
                     proj_b, bn2_g, bn2_b, bn2_m, bn2_v, fc1_w, fc1_b, dw_w,
                     fc2_w, fc2_b):
    f64 = np.float64
    inv1 = (bn1_g.astype(f64) / np.sqrt(bn1_v.astype(f64) + EPS))
    beta1 = bn1_b.astype(f64) - bn1_m.astype(f64) * inv1
    inv2 = (bn2_g.astype(f64) / np.sqrt(bn2_v.astype(f64) + EPS))
    beta2 = bn2_b.astype(f64) - bn2_m.astype(f64) * inv2

    tscale = np.repeat(temp.astype(f64), DH)                     # [256]
    qw_f = q_w.astype(f64) * inv1[None, :] * tscale[:, None]
    qb = (q_w.astype(f64) @ beta1) * tscale
    kw_f = k_w.astype(f64) * inv1[None, :]
    kb = k_w.astype(f64) @ beta1
    fc1w_f = fc1_w.astype(f64) * inv2[None, :]
    fc1bf = fc1_b.astype(f64) + fc1_w.astype(f64) @ beta2

    bf = np.dtype("bfloat16") if False else None  # placeholder
    import ml_dtypes
    bf16 = ml_dtypes.bfloat16

    dwd = np.zeros((8, 9, 128, 128), np.float32)
    idx = np.arange(128)
    for ct in range(8):
        for t in range(9):
            dy, dx = t // 3, t % 3
            dwd[ct, t, idx, idx] = dw_w[ct * 128 + idx, 0, dy, dx]

    w = {
        "qT": _to_part_layout(np.ascontiguousarray(qw_f.T).astype(np.float32), 2),
        "kT": _to_part_layout(np.ascontiguousarray(kw_f.T).astype(np.float32), 2),
        "vwT": _to_part_layout(np.ascontiguousarray(v_w.T).astype(np.float32), 2),
        "projT": _to_part_layout(np.ascontiguousarray(proj_w.T).astype(np.float32), 2),
        "fc1T": _to_part_layout(np.ascontiguousarray(fc1w_f.T).astype(np.float32), 2),
        "fc2T": _to_part_layout(np.ascontiguousarray(fc2_w.T).astype(np.float32), 8),
        "dwd": dwd,
        "qb": _bias_layout(qb.astype(np.float32), 2),
        "kb": _bias_layout(kb.astype(np.float32), 2),
        "projb": _bias_layout(proj_b.astype(np.float32), 2),
        "fc1b": _bias_layout(fc1bf.astype(np.float32), 8),
        "fc2b": _bias_layout(fc2_b.astype(np.float32), 2),
        "inv1": _bias_layout(inv1.astype(np.float32), 2),
        "beta1": _bias_layout(beta1.astype(np.float32), 2),
    }
    return w


_LAST_RESULTS = {}


def kernel(x1, x2, bn1_g, bn1_b, bn1_m, bn1_v, q_w, k_w, v_w, temp, proj_w,
           proj_b, bn2_g, bn2_b, bn2_m, bn2_v, fc1_w, fc1_b, dw_w, fc2_w,
           fc2_b, _trace=False):
    x1 = np.asarray(x1, np.float32)
    x2 = np.asarray(x2, np.float32)
    args = [np.asarray(a) for a in
            (bn1_g, bn1_b, bn1_m, bn1_v, q_w, k_w, v_w, temp, proj_w, proj_b,
             bn2_g, bn2_b, bn2_m, bn2_v, fc1_w, fc1_b, dw_w, fc2_w, fc2_b)]
    w = _prepare_weights(*args)

    nc = _build_nc()
    in_maps = []
    for i in range(N_CORES):
        m = dict(w)
        m["x1"] = _to_part_layout(x1[i].reshape(C, HW), 2)
        m["x2"] = _to_part_layout(x2[i].reshape(C, HW), 2)
        in_maps.append(m)

    res = run_bass_kernel_spmd(nc, in_maps, core_ids=list(range(N_CORES)),
                               trace=_trace)
    _LAST_RESULTS["res"] = res

    out = np.empty((B, C, H, W), np.float32)
    for i in range(N_CORES):
        o = res.results[i]["out"]                    # [128, 2, 1024]
        out[i] = o.transpose(1, 0, 2).reshape(C, H, W)
    return out


# revision 15
# speedup vs baseline: 1.3886x; 1.3886x over previous
"""CMXBlock (dense transformer block) Trainium2 Bass kernel.

Sharding: data-parallel over batch B=8 across the 8 NeuronCores — one image
per core, all weights replicated, no collectives.

Per-core computation (C=256 channels on partitions, HW=1024 positions free):
  x1 <- x1 + proj(softmax((q_w@bn1(x1))^T (k_w@bn1(x2)) * temp) @ (v_w@bn1(x2))^T)
  x1 <- x1 + fc2(gelu(dwconv3x3(fc1(bn2(x1)))))

Implementation notes:
 - BatchNorms are folded into the 1x1-conv weights host-side (bias applied
   per-partition at PSUM eviction).  temp is folded into q_w rows.
 - Matmuls run in float32r (full PE rate, ~1.8e-4 rel err) except the
   attention AV, depthwise conv and fc2 which run in bf16 (their moving
   operands are produced by ACT/DVE which cannot emit float32r).
 - Softmax: scores are built transposed (S^T[m,n] per m-tile), exp on ACT
   (no max subtraction — scores are bounded, fp32/bf16 range is ample), the
   denominator comes from an extra all-ones column in the AV stationary
   operand, and the 1/Z column scale is applied via a gpsimd
   partition-broadcast + one DVE multiply per channel half.
 - Depthwise 3x3 runs on the PE as 9 diagonal-matrix taps over an x-padded
   [32, 36] spatial layout so every shifted tap is a contiguous 1-D slice.
"""
import numpy as np

import concourse.bass as bass
import concourse.tile as tile
import concourse.mybir as mybir
from concourse import bacc
from concourse.bass_utils import run_bass_kernel_spmd

F32 = mybir.dt.float32
F32R = mybir.dt.float32r
BF16 = mybir.dt.bfloat16
FP16 = mybir.dt.float16
AF = mybir.ActivationFunctionType
ALU = mybir.AluOpType

B, C, H, W = 8, 256, 32, 32
NH, DH = 8, 32          # heads, head dim
HW = H * W              # 1024 positions
HID = 4 * C             # 1024 mlp hidden channels
EPS = 1e-5
WP = W + 4              # x-padded row width (36, even)
PADF = H * WP           # padded flat spatial size (1152)
N_CORES = 8

_NC_CACHE = {}


def _tap_chunks(shift):
    """Bank-aligned (<=512) even-aligned chunks of a dw tap's dst range.

    float32r matmuls need even dst offsets and even moving counts; the
    elements dropped by the even-alignment are always x-pad columns (never
    read downstream), and the full-coverage center tap has written them.
    """
    lo, hi = max(0, -shift), min(PADF, PADF - shift)
    out = []
    for b0 in range(0, PADF, 512):
        a, b = max(lo, b0), min(hi, b0 + 512)
        a += a % 2
        n = (b - a) & ~1
        if n > 0:
            out.append((a, n))
    return out


def _build_body(nc, tc, io):
    x1d, x2d = io["x1"], io["x2"]
    outd = io["out"]

    import contextlib
    ctx = contextlib.ExitStack()
    with ctx:
        wpool = ctx.enter_context(tc.tile_pool(name="weights", bufs=1))
        # pB: attention results that survive until after proj
        pB = ctx.enter_context(tc.tile_pool(name="pB", bufs=1))

        # ---------- persistent SBUF tensors ----------
        x1 = wpool.tile([128, 2, HW], F32R, tag="x1")
        nc.sync.dma_start(x1[:], x1d[:])

        def wload(name, shape, dt):
            t = wpool.tile(shape, dt, tag=name)
            nc.sync.dma_start(t[:], io[name][:])
            return t

        qT = wload("qT", [128, 2, C], F32R)
        kT = wload("kT", [128, 2, C], F32R)
        vwT = wload("vwT", [128, 2, C], F32R)
        projT = wload("projT", [128, 2, C], F32R)
        fc1T = wload("fc1T", [128, 2, HID], F32R)
        fc2T = wload("fc2T", [128, 8, C], F32R)
        qb = wload("qb", [128, 2], F32)
        kb = wload("kb", [128, 2], F32)
        projb = wload("projb", [128, 2], F32)
        fc1b = wload("fc1b", [128, 8], F32)
        fc2b = wload("fc2b", [128, 2], F32)
        inv1 = wload("inv1", [128, 2], F32)
        beta1 = wload("beta1", [128, 2], F32)

        attnU = pB.tile([128, 2, HW], F32, tag="attnU")
        z8 = pB.tile([NH, HW], F32, tag="z8")

        with tc.tile_pool(name="pA", bufs=1) as pA:
            q_sb = pA.tile([128, 2, HW], F32R, tag="q")
            k_sb = pA.tile([128, 2, HW], F32R, tag="k")
            vt1 = pA.tile([128, 8, NH, DH + 1], F32R, tag="vt1")  # [p, mt, h, v|1]
            # the all-ones denominator column must come from a DVE op (the
            # only engine the verifier accepts as a float32r producer)
            onesf = pA.tile([128, 8 * NH], F32, tag="onesf")
            nc.gpsimd.memset(onesf[:], 1.0)
            nc.vector.tensor_copy(
                vt1[:, :, :, DH:DH + 1],
                onesf[:].rearrange("p (a b c) -> p a b c", a=8, b=NH))

            # ---------- phase 1: q, k projections; x2n; v^T ----------
            with tc.tile_pool(name="p1", bufs=1) as p1, \
                 tc.tile_pool(name="ps1", bufs=2, space="PSUM") as ps1:
                x2 = p1.tile([128, 2, HW], F32R, tag="x2")
                nc.sync.dma_start(x2[:], x2d[:])
                for (wT, bias, dst) in ((qT, qb, q_sb), (kT, kb, k_sb)):
                    rhs = x1 if dst is q_sb else x2
                    for mt in range(2):
                        for chk in range(2):
                            ps = ps1.tile([128, 512], F32, tag="mm")
                            for kt in range(2):
                                nc.tensor.matmul(
                                    ps[:], wT[:, kt, 128 * mt:128 * (mt + 1)],
                                    rhs[:, kt, 512 * chk:512 * (chk + 1)],
                                    start=(kt == 0), stop=(kt == 1))
                            nc.vector.tensor_scalar_add(
                                dst[:, mt, 512 * chk:512 * (chk + 1)], ps[:],
                                bias[:, mt:mt + 1])

                x2n = p1.tile([128, 2, HW], F32R, tag="x2n")
                for kt in range(2):
                    nc.vector.tensor_scalar(
                        x2n[:, kt, :], x2[:, kt, :],
                        inv1[:, kt:kt + 1], beta1[:, kt:kt + 1], ALU.mult, ALU.add)

                for mp in range(8):
                    ps = ps1.tile([128, 512], F32, tag="mm")
                    for kt in range(2):
                        nc.tensor.matmul(
                            ps[:, 0:C], x2n[:, kt, 128 * mp:128 * (mp + 1)],
                            vwT[:, kt, :], start=(kt == 0), stop=(kt == 1))
                    nc.vector.tensor_copy(
                        vt1[:, mp, :, 0:DH],
                        ps[:, 0:C].rearrange("p (h d) -> p h d", h=NH))

            # ---------- phase 2: attention, 2 heads interleaved ----------
            # Two heads at different PE row groups run their K=32 score
            # matmuls concurrently; PSUM budget (8 banks) = 2 score tiles
            # (2 banks each) + 2 AV accumulators (2 banks each).
            with tc.tile_pool(name="ef", bufs=4) as epool, \
                 tc.tile_pool(name="expS", bufs=6) as xpool, \
                 tc.tile_pool(name="uz", bufs=2) as uz_pool, \
                 tc.tile_pool(name="pss", bufs=2, space="PSUM") as pss, \
                 tc.tile_pool(name="psa", bufs=2, space="PSUM") as psa:
                for half in range(2):
                    for pr in range(2):
                        ja, jb = 2 * pr, 2 * pr + 1
                        heads = [(half * 4 + ja, 32 * ja), (half * 4 + jb, 32 * jb)]
                        ps_av = {h: psa.tile([DH + 1, HW], F32, tag="av",
                                           name=f"ps_av_{h}")
                                 for h, _ in heads}
                        pend = {h: None for h, _ in heads}   # (mt, expS tile)
                        for mt in range(8):
                            ps_s = {h: pss.tile([128, HW], F32, tag="s",
                                              name=f"ps_s_{h}_{mt}")
                                    for h, _ in heads}
                            for chk in range(2):
                                for h, hb in heads:
                                    nc.tensor.matmul(
                                        ps_s[h][:, 512 * chk:512 * (chk + 1)],
                                        k_sb[hb:hb + 32, half, 128 * mt:128 * (mt + 1)],
                                        q_sb[hb:hb + 32, half, 512 * chk:512 * (chk + 1)],
                                        start=True, stop=True, tile_position=(hb, 0))
                            for h, hb in heads:
                                ef = epool.tile([128, HW], F32, tag="ef")
                                nc.scalar.activation(ef[:], ps_s[h][:], AF.Exp)
                                ex = xpool.tile([128, HW], F32R, tag="expS")
                                nc.vector.tensor_copy(ex[:], ef[:])
                                if pend[h] is not None:
                                    pmt, pex = pend[h]
                                    for chk in range(2):
                                        nc.tensor.matmul(
                                            ps_av[h][:, 512 * chk:512 * (chk + 1)],
                                            vt1[:, pmt, h, :],
                                            pex[:, 512 * chk:512 * (chk + 1)],
                                            start=(pmt == 0), stop=False)
                                pend[h] = (mt, ex)
                        for h, hb in heads:
                            pmt, pex = pend[h]
                            for chk in range(2):
                                nc.tensor.matmul(
                                    ps_av[h][:, 512 * chk:512 * (chk + 1)],
                                    vt1[:, pmt, h, :],
                                    pex[:, 512 * chk:512 * (chk + 1)],
                                    start=False, stop=True)
                            uz = uz_pool.tile([DH + 1, HW], F32, tag="uz")
                            nc.vector.tensor_copy(uz[:], ps_av[h][:])
                            nc.sync.dma_start(z8[h:h + 1, :], uz[DH:DH + 1, :])
                            nc.sync.dma_start(attnU[hb:hb + 32, half, :], uz[0:DH, :])

        # ---------- phase 3: normalize, proj, residual ----------
        r8 = pB.tile([NH, HW], F32, tag="r8")
        nc.vector.reciprocal(r8[:], z8[:])
        rbc = pB.tile([128, 2, HW], F32, tag="rbc")
        with tc.tile_pool(name="rtmp", bufs=2) as rpool:
            for h in range(NH):
                hb, half = 32 * (h % 4), h // 4
                rt = rpool.tile([1, HW], F32, tag="rt")
                nc.sync.dma_start(rt[:], r8[h:h + 1, :])
                # partition_broadcast always writes from partition 0, so
                # broadcast into a base-0 temp and DMA to the head's rows.
                rb = rpool.tile([32, HW], F32, tag="rb")
                nc.gpsimd.partition_broadcast(rb[:], rt[:])
                nc.sync.dma_start(rbc[hb:hb + 32, half, :], rb[:])
        attn_r = pB.tile([128, 2, HW], F32R, tag="attn_r")
        for half in range(2):
            nc.vector.tensor_mul(attn_r[:, half, :], attnU[:, half, :],
                                 rbc[:, half, :])

        with tc.tile_pool(name="pC", bufs=1) as pC, \
             tc.tile_pool(name="ps2", bufs=2, space="PSUM") as ps2, \
             tc.tile_pool(name="psd", bufs=2, space="PSUM") as psd:
            x1u = pC.tile([128, 2, HW], F32R, tag="x1u")
            h1 = pC.tile([128, 8, H, WP], FP16, tag="h1")
            # pad columns must come from a DVE op (f32r-producer whitelist)
            zpad = pC.tile([128, 8 * H * 2], F32, tag="zpad")
            nc.gpsimd.memset(zpad[:], 0.0)
            zsrc = zpad[:].rearrange("p (c a b) -> p c a b", c=8, a=H)
            nc.vector.tensor_copy(h1[:, :, :, 0:2], zsrc)
            nc.vector.tensor_copy(h1[:, :, :, WP - 2:WP], zsrc)
            hgr = pC.tile([128, 8, HW], F32R, tag="hgr")
            out_sb = pC.tile([128, 2, HW], F32, tag="out")

            # proj + residual1
            for mt in range(2):
                for chk in range(2):
                    ps = ps2.tile([128, 512], F32, tag="mm")
                    for kt in range(2):
                        nc.tensor.matmul(
                            ps[:], projT[:, kt, 128 * mt:128 * (mt + 1)],
                            attn_r[:, kt, 512 * chk:512 * (chk + 1)],
                            start=(kt == 0), stop=(kt == 1))
                    nc.vector.scalar_tensor_tensor(
                        x1u[:, mt, 512 * chk:512 * (chk + 1)], ps[:],
                        projb[:, mt:mt + 1],
                        x1[:, mt, 512 * chk:512 * (chk + 1)],
                        ALU.add, ALU.add)

            # ---------- phase 4: MLP ----------
            for mt in range(8):
                for chk in range(2):
                    ps = ps2.tile([128, 512], F32, tag="mm")
                    for kt in range(2):
                        nc.tensor.matmul(
                            ps[:], fc1T[:, kt, 128 * mt:128 * (mt + 1)],
                            x1u[:, kt, 512 * chk:512 * (chk + 1)],
                            start=(kt == 0), stop=(kt == 1))
                    nc.vector.tensor_scalar_add(
                        h1[:, mt, 16 * chk:16 * (chk + 1), 2:W + 2],
                        ps[:].rearrange("p (a b) -> p a b", a=16),
                        fc1b[:, mt:mt + 1])

            # depthwise 3x3 (9 diagonal taps, f32r, diag tiles streamed) + gelu
            h1f = h1[:].rearrange("p c a b -> p c (a b)")
            taps = [(dy, dx) for dy in (-1, 0, 1) for dx in (-1, 0, 1)]
            taps.remove((0, 0))
            taps = [(0, 0)] + taps      # center first: full coverage, start=True
            with tc.tile_pool(name="dwd", bufs=4) as dpool, \
                 tc.tile_pool(name="hgf", bufs=2) as gpool:
                for ct in range(8):
                    ps_dw = psd.tile([128, PADF], F32, tag="dw")
                    for ti, (dy, dx) in enumerate(taps):
                        dwt = dpool.tile([128, 128], FP16, tag="dwt")
                        nc.sync.dma_start(
                            dwt[:], io["dwd"][ct, 3 * (dy + 1) + (dx + 1)])
                        shift = dy * WP + dx
                        chunks = _tap_chunks(shift)
                        for ci, (c0, n) in enumerate(chunks):
                            nc.tensor.matmul(
                                ps_dw[:, c0:c0 + n],
                                dwt[:],
                                h1f[:, ct, c0 + shift:c0 + shift + n],
                                start=(ti == 0),
                                stop=(ti == len(taps) - 1 and ci == len(chunks) - 1))
                    hgf = gpool.tile([128, HW], F32, tag="hgf")
                    nc.scalar.activation(
                        hgf[:],
                        ps_dw[:].rearrange("p (a b) -> p a b", a=H)[:, :, 2:W + 2],
                        AF.Gelu)
                    nc.vector.tensor_copy(hgr[:, ct, :], hgf[:])

            # fc2 + residual2
            for mt in range(2):
                for chk in range(2):
                    ps = ps2.tile([128, 512], F32, tag="mm")
                    for kt in range(8):
                        nc.tensor.matmul(
                            ps[:], fc2T[:, kt, 128 * mt:128 * (mt + 1)],
                            hgr[:, kt, 512 * chk:512 * (chk + 1)],
                            start=(kt == 0), stop=(kt == 7))
                    nc.vector.scalar_tensor_tensor(
                        out_sb[:, mt, 512 * chk:512 * (chk + 1)], ps[:],
                        fc2b[:, mt:mt + 1],
                        x1u[:, mt, 512 * chk:512 * (chk + 1)],
                        ALU.add, ALU.add)

            nc.sync.dma_start(outd[:], out_sb[:])


def _build_nc():
    if "nc" in _NC_CACHE:
        return _NC_CACHE["nc"]
    nc = bacc.Bacc(trn_type="TRN2", target_bir_lowering=False, debug=False)
    io = {}
    for name, shape, dt in [
        ("x1", [128, 2, HW], F32R), ("x2", [128, 2, HW], F32R),
        ("qT", [128, 2, C], F32R), ("kT", [128, 2, C], F32R),
        ("vwT", [128, 2, C], F32R), ("projT", [128, 2, C], F32R),
        ("fc1T", [128, 2, HID], F32R), ("fc2T", [128, 8, C], F32R),
        ("dwd", [8, 9, 128, 128], FP16),
        ("qb", [128, 2], F32), ("kb", [128, 2], F32), ("projb", [128, 2], F32),
        ("fc1b", [128, 8], F32), ("fc2b", [128, 2], F32),
        ("inv1", [128, 2], F32), ("beta1", [128, 2], F32),
    ]:
        io[name] = nc.dram_tensor(name, shape, dt, kind="ExternalInput").ap()
    io["out"] = nc.dram_tensor("out", [128, 2, HW], F32, kind="ExternalOutput").ap()

    with tile.TileContext(nc) as tc:
        _build_body(nc, tc, io)
    nc.compile()
    _NC_CACHE["nc"] = nc
    return nc


def _to_part_layout(a, ntiles):
    """[ntiles*128, F] -> [128, ntiles, F] with c = kt*128 + p."""
    return np.ascontiguousarray(
        a.reshape(ntiles, 128, -1).transpose(1, 0, 2))


def _bias_layout(b, ntiles):
    """[ntiles*128] -> [128, ntiles]."""
    return np.ascontiguousarray(b.reshape(ntiles, 128).T)


def _prepare_weights(bn1_g, bn1_b, bn1_m, bn1_v, q_w, k_w, v_w, temp, proj_w,
                     proj_b, bn2_g, bn2_b, bn2_m, bn2_v, fc1_w, fc1_b, dw_w,
                     fc2_w, fc2_b):
    f64 = np.float64
    inv1 = (bn1_g.astype(f64) / np.sqrt(bn1_v.astype(f64) + EPS))
    beta1 = bn1_b.astype(f64) - bn1_m.astype(f64) * inv1
    inv2 = (bn2_g.astype(f64) / np.sqrt(bn2_v.astype(f64) + EPS))
    beta2 = bn2_b.astype(f64) - bn2_m.astype(f64) * inv2

    tscale = np.repeat(temp.astype(f64), DH)                     # [256]
    qw_f = q_w.astype(f64) * inv1[None, :] * tscale[:, None]
    qb = (q_w.astype(f64) @ beta1) * tscale
    kw_f = k_w.astype(f64) * inv1[None, :]
    kb = k_w.astype(f64) @ beta1
    fc1w_f = fc1_w.astype(f64) * inv2[None, :]
    fc1bf = fc1_b.astype(f64) + fc1_w.astype(f64) @ beta2

    bf = np.dtype("bfloat16") if False else None  # placeholder
    import ml_dtypes
    bf16 = ml_dtypes.bfloat16

    dwd = np.zeros((8, 9, 128, 128), np.float32)
    idx = np.arange(128)
    for ct in range(8):
        for t in range(9):
            dy, dx = t // 3, t % 3
            dwd[ct, t, idx, idx] = dw_w[ct * 128 + idx, 0, dy, dx]

    w = {
        "qT": _to_part_layout(np.ascontiguousarray(qw_f.T).astype(np.float32), 2),
        "kT": _to_part_layout(np.ascontiguousarray(kw_f.T).astype(np.float32), 2),
        "vwT": _to_part_layout(np.ascontiguousarray(v_w.T).astype(np.float32), 2),
        "projT": _to_part_layout(np.ascontiguousarray(proj_w.T).astype(np.float32), 2),
        "fc1T": _to_part_layout(np.ascontiguousarray(fc1w_f.T).astype(np.float32), 2),
        "fc2T": _to_part_layout(np.ascontiguousarray(fc2_w.T).astype(np.float32), 8),
        "dwd": dwd.astype(np.float16),
        "qb": _bias_layout(qb.astype(np.float32), 2),
        "kb": _bias_layout(kb.astype(np.float32), 2),
        "projb": _bias_layout(proj_b.astype(np.float32), 2),
        "fc1b": _bias_layout(fc1bf.astype(np.float32), 8),
        "fc2b": _bias_layout(fc2_b.astype(np.float32), 2),
        "inv1": _bias_layout(inv1.astype(np.float32), 2),
        "beta1": _bias_layout(beta1.astype(np.float32), 2),
    }
    return w


_LAST_RESULTS = {}


def kernel(x1, x2, bn1_g, bn1_b, bn1_m, bn1_v, q_w, k_w, v_w, temp, proj_w,
           proj_b, bn2_g, bn2_b, bn2_m, bn2_v, fc1_w, fc1_b, dw_w, fc2_w,
           fc2_b, _trace=False):
    x1 = np.asarray(x1, np.float32)
    x2 = np.asarray(x2, np.float32)
    args = [np.asarray(a) for a in
            (bn1_g, bn1_b, bn1_m, bn1_v, q_w, k_w, v_w, temp, proj_w, proj_b,
             bn2_g, bn2_b, bn2_m, bn2_v, fc1_w, fc1_b, dw_w, fc2_w, fc2_b)]
    w = _prepare_weights(*args)

    nc = _build_nc()
    in_maps = []
    for i in range(N_CORES):
        m = dict(w)
        m["x1"] = _to_part_layout(x1[i].reshape(C, HW), 2)
        m["x2"] = _to_part_layout(x2[i].reshape(C, HW), 2)
        in_maps.append(m)

    res = run_bass_kernel_spmd(nc, in_maps, core_ids=list(range(N_CORES)),
                               trace=_trace)
    _LAST_RESULTS["res"] = res

    out = np.empty((B, C, H, W), np.float32)
    for i in range(N_CORES):
        o = res.results[i]["out"]                    # [128, 2, 1024]
        out[i] = o.transpose(1, 0, 2).reshape(C, H, W)
    return out
